# revision 1
# baseline (speedup 1.0000x reference)
"""Trainium2 Bass kernel for nn_LocalPoolNet (3x SAGEConv + TopKPool + readout + MLP).

Strategy:
- Shard B=100 graphs across 8 cores (padded to 104 = 8*13).
- Host prep: per-graph dense adjacency A^T (bf16; integer counts are exact),
  node features padded to 512 slots/graph with a keep column.
- Device: everything else. Convs as dense per-graph matmuls (agg = A @ h with
  the keep column giving cnt for free), per-node scalings applied in
  node-major layout around PE transposes, top-k thresholds via a batched
  16-way bisection on DVE over a [13 graphs, 512 nodes] score tile, masked
  readouts, MLP + log_softmax on chip.
- All pools/readouts computed in original (non-compacted) node coordinates
  with keep masks; readouts and selections are order-invariant so this
  matches the reference exactly.
"""
import os
import sys

sys.path.insert(0, "/opt/trn_rl_repo")

import numpy as np
import ml_dtypes

import concourse.bass as bass
import concourse.tile as tile
from concourse import mybir
from concourse.bass_utils import run_bass_kernel_spmd
from bass_rust import ScopedClock

F32 = mybir.dt.float32
BF16 = mybir.dt.bfloat16
AF = mybir.ActivationFunctionType
ALU = mybir.AluOpType
AX = mybir.AxisListType

B, NPG, DEG = 100, 500, 12
F, C = 128, 10
P = 512           # padded nodes per graph
NCH = 4           # 512 / 128
GPC = 13          # graphs per core
NCORES = 8
BPAD = GPC * NCORES  # 104
KS = [250, 125, 63]
NW = 16           # bisection fan-out
NROUNDS = 11      # (2.0002) * 16^-11 ~ 1.2e-13 resolution

LAST_EXEC_NS = None


class PatchedTileContext(tile.TileContext):
    """This walrus build allows only one sync-wait per CTRL instruction; the
    stock Tile kernel-tail drain aggregates one wait per live sem. Split the
    waits across single-wait nops in front of the drain."""

    def _drain_and_barrier(self, tick_clock, wait_clock):
        probe = self.nc.sync.nop(nofuse=True)
        wait_clock.add_sem_waits(
            probe.ins, ScopedClock({None: tick_clock.global_clock})
        )
        waits = list(probe.ins.sync_info.on_wait or [])
        probe.ins.sync_info.on_wait = waits[:1]
        for w in waits[1:]:
            n2 = self.nc.sync.nop(nofuse=True)
            n2.ins.sync_info = mybir.SyncInfo(on_wait=[w], on_update=[])
        self.nc.sync.drain()
        self.nc.all_engine_barrier()
        assert self.sems is not None
        popped = self.nc._tile_sem_poison_stack.pop()
        assert popped is self._sem_poison
        self.nc.clear_and_free_semaphores(list(self.sems.allocated().values()))
        self.nc.all_engine_barrier()


def split_sync_waits(nc, limit=1):
    """This walrus build rejects instructions carrying more than one sync
    wait; hoist extras onto same-engine NOPs placed immediately before."""
    n = 0
    for f in nc.m.functions:
        for bb in f.blocks:
            insts = bb.instructions
            out = []
            for inst in insts:
                si = inst.sync_info
                waits = list(si.on_wait) if si and si.on_wait else []
                if len(waits) > limit:
                    for w in waits[:-limit] if limit else waits:
                        nop = mybir.InstNoOp(name=f"wsplit_{n}",
                                             engine=inst.engine)
                        n += 1
                        nop.sync_info = mybir.SyncInfo(on_wait=[w],
                                                       on_update=[])
                        out.append(nop)
                    si.on_wait = waits[-limit:] if limit else []
                out.append(inst)
            insts[:] = out


def build_nc(scales):
    """scales[l] = 1/||pw_l||."""
    nc = bass.Bass("TRN2", target_bir_lowering=False, debug=False,
                   num_devices=NCORES)

    xn_d = nc.dram_tensor("xn", [GPC, P, 132], F32, kind="ExternalInput")
    at_d = nc.dram_tensor("at", [GPC, P, P], BF16, kind="ExternalInput")
    idn_d = nc.dram_tensor("idn", [128, 128], F32, kind="ExternalInput")
    biota_d = nc.dram_tensor("biota", [16, NW], F32, kind="ExternalInput")
    wl_d = [nc.dram_tensor(f"wl{l}", [128, 128], F32, kind="ExternalInput") for l in range(3)]
    wr_d = [nc.dram_tensor(f"wr{l}", [128, 128], F32, kind="ExternalInput") for l in range(3)]
    bl_d = [nc.dram_tensor(f"bl{l}", [128, 1], F32, kind="ExternalInput") for l in range(3)]
    pw_d = [nc.dram_tensor(f"pw{l}", [128, 1], F32, kind="ExternalInput") for l in range(3)]
    w1a_d = nc.dram_tensor("w1a", [128, 128], F32, kind="ExternalInput")
    w1b_d = nc.dram_tensor("w1b", [128, 128], F32, kind="ExternalInput")
    b1_d = nc.dram_tensor("b1", [128, 1], F32, kind="ExternalInput")
    w2_d = nc.dram_tensor("w2", [128, 64], F32, kind="ExternalInput")
    b2_d = nc.dram_tensor("b2", [64, 1], F32, kind="ExternalInput")
    w3_d = nc.dram_tensor("w3", [64, 10], F32, kind="ExternalInput")
    b3_d = nc.dram_tensor("b3r", [16, 10], F32, kind="ExternalInput")
    out_d = nc.dram_tensor("out", [GPC, 10], F32, kind="ExternalOutput")
    scores_dram = nc.dram_tensor("scores_scratch", [GPC, P], F32)
    keep_dram = nc.dram_tensor("keep_scratch", [GPC, P], F32)

    with PatchedTileContext(nc) as tc:
        cpool = tc.alloc_tile_pool(name="consts", bufs=1)
        idn = cpool.tile([128, 128], F32, tag="idn")
        nc.sync.dma_start(idn[:], idn_d.ap())
        biota = cpool.tile([16, NW], F32, tag="biota")
        nc.sync.dma_start(biota[:], biota_d.ap())
        wl = [cpool.tile([128, 128], F32, tag=f"wl{l}", name=f"wl{l}") for l in range(3)]
        wr = [cpool.tile([128, 128], F32, tag=f"wr{l}", name=f"wr{l}") for l in range(3)]
        bl = [cpool.tile([128, 1], F32, tag=f"bl{l}", name=f"bl{l}") for l in range(3)]
        pw = [cpool.tile([128, 1], F32, tag=f"pw{l}", name=f"pw{l}") for l in range(3)]
        for l in range(3):
            nc.sync.dma_start(wl[l][:], wl_d[l].ap())
            nc.sync.dma_start(wr[l][:], wr_d[l].ap())
            nc.sync.dma_start(bl[l][:], bl_d[l].ap())
            nc.sync.dma_start(pw[l][:], pw_d[l].ap())
        w1a = cpool.tile([128, 128], F32, tag="w1a")
        w1b = cpool.tile([128, 128], F32, tag="w1b")
        b1 = cpool.tile([128, 1], F32, tag="b1")
        w2 = cpool.tile([128, 64], F32, tag="w2")
        b2 = cpool.tile([64, 1], F32, tag="b2")
        w3 = cpool.tile([64, 10], F32, tag="w3")
        b3r = cpool.tile([16, 10], F32, tag="b3r")
        for t, d in [(w1a, w1a_d), (w1b, w1b_d), (b1, b1_d), (w2, w2_d),
                     (b2, b2_d), (w3, w3_d), (b3r, b3_d)]:
            nc.sync.dma_start(t[:], d.ap())

        # resident state
        big = tc.alloc_tile_pool(name="big", bufs=1)
        atall = big.tile([128, GPC, NCH, P], BF16, tag="atall")
        hiall = big.tile([128, GPC, NCH, 132], BF16, tag="hiall")
        loall = big.tile([128, GPC, NCH, 132], BF16, tag="loall")
        htall = big.tile([128, GPC, P], F32, tag="htall")
        negmall = big.tile([128, GPC, NCH], F32, tag="negmall")
        scoresT = big.tile([16, P], F32, tag="scoresT")
        keepT = big.tile([16, P], F32, tag="keepT")
        rdMax = [big.tile([128, GPC], F32, tag=f"rmax{l}", name=f"rmax{l}") for l in range(3)]
        rdMean = [big.tile([128, GPC], F32, tag=f"rmean{l}", name=f"rmean{l}") for l in range(3)]
        # bisection state
        lo13 = big.tile([16, 1], F32, tag="lo13")
        st13 = big.tile([16, 1], F32, tag="st13")
        t16 = big.tile([16, NW], F32, tag="t16")
        cmp = big.tile([16, NW, P], F32, tag="cmp")
        cnts = big.tile([16, NW], F32, tag="cnts")
        flags = big.tile([16, NW], F32, tag="flags")
        jj = big.tile([16, 1], F32, tag="jj")
        dl = big.tile([16, 1], F32, tag="dl")

        work = tc.alloc_tile_pool(name="work", bufs=2)
        psA = tc.alloc_tile_pool(name="psA", bufs=2, space="PSUM")
        psT = tc.alloc_tile_pool(name="psT", bufs=3, space="PSUM")
        psZ = tc.alloc_tile_pool(name="psZ", bufs=1, space="PSUM")
        psU = tc.alloc_tile_pool(name="psU", bufs=1, space="PSUM")

        G = GPC
        # ---- load + level-1 prep ----
        for g in range(G):
            nc.sync.dma_start(
                atall[:, g], at_d.ap()[g].rearrange("(c p) d -> p c d", p=128))
            xn = work.tile([128, NCH, 132], F32, tag="xn")
            nc.sync.dma_start(
                xn[:], xn_d.ap()[g].rearrange("(c p) f -> p c f", p=128))
            nc.scalar.copy(hiall[:, g, :, 0:129], xn[:, :, 0:129])
            nc.vector.tensor_tensor(loall[:, g, :, 0:129], xn[:, :, 0:129],
                                    hiall[:, g, :, 0:129], ALU.subtract)
            ps_hT = psT.tile([128, P], F32, tag="psT")
            for c in range(NCH):
                nc.tensor.matmul(ps_hT[:, c * 128:(c + 1) * 128],
                                 xn[:, c, 0:128], idn[:],
                                 is_transpose=True, skip_group_check=True)
            nc.scalar.copy(htall[:, g], ps_hT[:])
            nc.vector.tensor_scalar(negmall[:, g], xn[:, :, 128], 1.0, 1.0e30,
                                    ALU.subtract, ALU.mult)

        for l in range(3):
            # ---- convs + scores ----
            for g in range(G):
                aggN = work.tile([128, NCH, 132], F32, tag="aggN")
                for dc in range(NCH):
                    ps_ag = psA.tile([128, 132], F32, tag="psA")
                    for j, srct in enumerate((hiall, loall)):
                        for sc in range(NCH):
                            nc.tensor.matmul(
                                ps_ag[:, 0:129],
                                atall[:, g, sc, dc * 128:(dc + 1) * 128],
                                srct[:, g, sc, 0:129],
                                start=(j == 0 and sc == 0),
                                stop=(j == 1 and sc == NCH - 1))
                    nc.scalar.copy(aggN[:, dc, 0:129], ps_ag[:, 0:129])

                cntm = work.tile([128, NCH], F32, tag="cntm")
                nc.vector.tensor_scalar_max(cntm[:], aggN[:, :, 128], 1.0)
                rN = work.tile([128, NCH], F32, tag="rN")
                nc.vector.reciprocal(rN[:], cntm[:])
                ee = work.tile([128, NCH], F32, tag="ee")
                nc.vector.tensor_tensor(ee[:], cntm[:], rN[:], ALU.mult)
                nc.vector.tensor_scalar(ee[:], ee[:], -1.0, 2.0,
                                        ALU.mult, ALU.add)
                nc.vector.tensor_tensor(rN[:], rN[:], ee[:], ALU.mult)
                meanN = work.tile([128, NCH, 128], F32, tag="meanN")
                for dc in range(NCH):
                    nc.vector.tensor_scalar_mul(meanN[:, dc],
                                                aggN[:, dc, 0:128],
                                                rN[:, dc:dc + 1])
                ps_mT = psT.tile([128, P], F32, tag="psT")
                for dc in range(NCH):
                    nc.tensor.matmul(ps_mT[:, dc * 128:(dc + 1) * 128],
                                     meanN[:, dc], idn[:], is_transpose=True,
                                     skip_group_check=True)
                meanT = work.tile([128, P], F32, tag="meanT")
                nc.scalar.copy(meanT[:], ps_mT[:])

                ps_z = psZ.tile([128, P], F32, tag="psZ")
                nc.tensor.matmul(ps_z[:], wl[l][:], meanT[:],
                                 start=True, stop=False)
                nc.tensor.matmul(ps_z[:], wr[l][:], htall[:, g],
                                 start=False, stop=True)
                nc.scalar.activation(htall[:, g], ps_z[:], AF.Relu,
                                     bias=bl[l][:])

                ps_u = psU.tile([1, P], F32, tag="psU")
                nc.tensor.matmul(ps_u[:], pw[l][:], htall[:, g])
                urow = work.tile([1, P], F32, tag="urow")
                nc.scalar.activation(urow[:], ps_u[:], AF.Tanh,
                                     scale=float(scales[l]))
                nc.sync.dma_start(scores_dram.ap()[g:g + 1, :], urow[:])

            # ---- batched 16-way bisection over [GPC, P] scores ----
            nc.sync.dma_start(scoresT[0:GPC, :], scores_dram.ap())
            nc.vector.memset(lo13[0:GPC, :], -1.0001)
            nc.vector.memset(st13[0:GPC, :], 2.0002 / NW)
            nc.vector.tensor_scalar(t16[0:GPC, :], biota[0:GPC, :],
                                    st13[0:GPC, 0:1], lo13[0:GPC, 0:1],
                                    ALU.mult, ALU.add)
            for r in range(NROUNDS):
                nc.vector.tensor_tensor(
                    cmp[0:GPC],
                    scoresT[0:GPC].unsqueeze(1).broadcast_to((GPC, NW, P)),
                    t16[0:GPC].unsqueeze(2).broadcast_to((GPC, NW, P)),
                    ALU.is_ge)
                nc.vector.tensor_reduce(cnts[0:GPC, :], cmp[0:GPC], AX.X,
                                        ALU.add)
                nc.vector.tensor_scalar(flags[0:GPC, :], cnts[0:GPC, :],
                                        float(KS[l]), None, ALU.is_ge)
                nc.vector.tensor_reduce(jj[0:GPC, :], flags[0:GPC, :], AX.X,
                                        ALU.add)
                nc.vector.tensor_scalar_sub(jj[0:GPC, :], jj[0:GPC, :], 1.0)
                nc.vector.tensor_tensor(dl[0:GPC, :], jj[0:GPC, :],
                                        st13[0:GPC, :], ALU.mult)
                nc.vector.tensor_tensor(lo13[0:GPC, :], lo13[0:GPC, :],
                                        dl[0:GPC, :], ALU.add)
                nc.vector.tensor_scalar_mul(st13[0:GPC, :], st13[0:GPC, :],
                                            1.0 / NW)
                if r < NROUNDS - 1:
                    nc.vector.tensor_scalar(t16[0:GPC, :], biota[0:GPC, :],
                                            st13[0:GPC, 0:1],
                                            lo13[0:GPC, 0:1],
                                            ALU.mult, ALU.add)
            nc.vector.tensor_scalar(keepT[0:GPC, :], scoresT[0:GPC, :],
                                    lo13[0:GPC, 0:1], None, ALU.is_ge)
            nc.sync.dma_start(keep_dram.ap(), keepT[0:GPC, :])

            # ---- pool epilogue + readouts ----
            for g in range(G):
                keepN = work.tile([128, NCH], F32, tag="keepN")
                nc.sync.dma_start(
                    keepN[:],
                    keep_dram.ap()[g].rearrange("(c p) -> p c", p=128))
                scoreN = work.tile([128, NCH], F32, tag="scoreN")
                nc.sync.dma_start(
                    scoreN[:],
                    scores_dram.ap()[g].rearrange("(c p) -> p c", p=128))
                vN = work.tile([128, NCH], F32, tag="vN")
                nc.vector.tensor_tensor(vN[:], scoreN[:], keepN[:], ALU.mult)
                nc.vector.tensor_scalar(negmall[:, g], keepN[:], 1.0, 1.0e30,
                                        ALU.subtract, ALU.mult)

                ps_hN = psT.tile([128, P], F32, tag="psT")
                for c in range(NCH):
                    nc.tensor.matmul(ps_hN[:, c * 128:(c + 1) * 128],
                                     htall[:, g, c * 128:(c + 1) * 128],
                                     idn[:], is_transpose=True,
                                     skip_group_check=True)
                tmp = work.tile([128, NCH, 128], F32, tag="tmp")
                for c in range(NCH):
                    nc.scalar.activation(tmp[:, c],
                                         ps_hN[:, c * 128:(c + 1) * 128],
                                         AF.Copy, scale=vN[:, c:c + 1])
                if l < 2:
                    nc.scalar.copy(hiall[:, g, :, 0:128], tmp[:])
                    nc.vector.tensor_copy(hiall[:, g, :, 128], keepN[:])
                    nc.vector.tensor_tensor(loall[:, g, :, 0:128], tmp[:],
                                            hiall[:, g, :, 0:128],
                                            ALU.subtract)
                    nc.vector.memset(loall[:, g, :, 128], 0.0)

                maskedN = work.tile([128, NCH, 128], F32, tag="maskedN")
                for c in range(NCH):
                    nc.vector.tensor_scalar_add(maskedN[:, c], tmp[:, c],
                                                negmall[:, g, c:c + 1])

                ps_hpT = psT.tile([128, P], F32, tag="psT")
                for c in range(NCH):
                    nc.tensor.matmul(ps_hpT[:, c * 128:(c + 1) * 128],
                                     tmp[:, c], idn[:], is_transpose=True,
                                     skip_group_check=True)
                nc.scalar.copy(htall[:, g], ps_hpT[:])
                nc.vector.tensor_reduce(rdMean[l][:, g:g + 1], htall[:, g],
                                        AX.X, ALU.add)

                ps_mkT = psT.tile([128, P], F32, tag="psT")
                for c in range(NCH):
                    nc.tensor.matmul(ps_mkT[:, c * 128:(c + 1) * 128],
                                     maskedN[:, c], idn[:], is_transpose=True,
                                     skip_group_check=True)
                nc.vector.tensor_reduce(rdMax[l][:, g:g + 1], ps_mkT[:],
                                        AX.X, ALU.max)

        # ---- z = sum_l readouts; MLP; log_softmax ----
        mlp = tc.alloc_tile_pool(name="mlp", bufs=1)
        zmax = mlp.tile([128, GPC], F32, tag="zmax")
        nc.vector.tensor_tensor(zmax[:], rdMax[0][:], rdMax[1][:], ALU.add)
        nc.vector.tensor_tensor(zmax[:], zmax[:], rdMax[2][:], ALU.add)
        zmean = mlp.tile([128, GPC], F32, tag="zmean")
        nc.vector.tensor_scalar_mul(rdMean[0][:], rdMean[0][:], 1.0 / KS[0])
        nc.vector.tensor_scalar_mul(rdMean[1][:], rdMean[1][:], 1.0 / KS[1])
        nc.vector.tensor_scalar_mul(rdMean[2][:], rdMean[2][:], 1.0 / KS[2])
        nc.vector.tensor_tensor(zmean[:], rdMean[0][:], rdMean[1][:], ALU.add)
        nc.vector.tensor_tensor(zmean[:], zmean[:], rdMean[2][:], ALU.add)

        ps_a1 = psZ.tile([128, GPC], F32, tag="psZ")
        nc.tensor.matmul(ps_a1[:], w1a[:], zmax[:], start=True, stop=False)
        nc.tensor.matmul(ps_a1[:], w1b[:], zmean[:], start=False, stop=True)
        a1 = mlp.tile([128, GPC], F32, tag="a1")
        nc.scalar.activation(a1[:], ps_a1[:], AF.Relu, bias=b1[:])

        ps_a2 = psZ.tile([64, GPC], F32, tag="psZ")
        nc.tensor.matmul(ps_a2[:], w2[:], a1[:])
        a2 = mlp.tile([64, GPC], F32, tag="a2")
        nc.scalar.activation(a2[:], ps_a2[:], AF.Relu, bias=b2[:])

        ps_o = psZ.tile([GPC, 10], F32, tag="psZ")
        nc.tensor.matmul(ps_o[:], a2[:], w3[:])
        o = mlp.tile([GPC, 10], F32, tag="o")
        nc.vector.tensor_tensor(o[:], ps_o[:], b3r[0:GPC, :], ALU.add)

        mx = mlp.tile([GPC, 1], F32, tag="mx")
        nc.vector.tensor_reduce(mx[:], o[:], AX.X, ALU.max)
        om = mlp.tile([GPC, 10], F32, tag="om")
        nc.vector.tensor_scalar_sub(om[:], o[:], mx[:, 0:1])
        ex = mlp.tile([GPC, 10], F32, tag="ex")
        nc.scalar.activation(ex[:], om[:], AF.Exp)
        sm = mlp.tile([GPC, 1], F32, tag="sm")
        nc.vector.tensor_reduce(sm[:], ex[:], AX.X, ALU.add)
        lse = mlp.tile([GPC, 1], F32, tag="lse")
        nc.scalar.activation(lse[:], sm[:], AF.Ln)
        res = mlp.tile([GPC, 10], F32, tag="res")
        nc.vector.tensor_scalar_sub(res[:], om[:], lse[:, 0:1])
        nc.sync.dma_start(out_d.ap(), res[:])

        for p in (mlp, psU, psZ, psT, psA, work, big, cpool):
            p.release()

    split_sync_waits(nc)
    return nc


def prep_inputs(x, edge_index):
    x = np.ascontiguousarray(np.asarray(x, np.float32))
    ei = np.asarray(edge_index, np.int64)
    src, dst = ei[0], ei[1]

    xn = np.zeros((BPAD, P, 132), np.float32)
    xn[:B, :NPG, 0:128] = x.reshape(B, NPG, 128)
    xn[:B, :NPG, 128] = 1.0
    xn[B:] = xn[B - (BPAD - B):B]

    g = src // NPG
    s = src - g * NPG
    d = dst - (dst // NPG) * NPG
    flat = (g * P + s) * P + d
    counts = np.bincount(flat, minlength=B * P * P).reshape(B, P, P)
    assert counts.max() < 256, counts.max()
    at = np.zeros((BPAD, P, P), ml_dtypes.bfloat16)
    at[:B] = counts.astype(ml_dtypes.bfloat16)
    at[B:] = at[B - (BPAD - B):B]
    return xn, at


_CACHE = {}


def kernel(**inputs):
    global LAST_EXEC_NS
    x = np.asarray(inputs["x"], np.float32)
    edge_index = np.asarray(inputs["edge_index"], np.int32)
    pws = [np.asarray(inputs[f"pw{l+1}"], np.float32) for l in range(3)]

    xn, at = prep_inputs(x, edge_index)
    scales = [1.0 / np.linalg.norm(pws[l]) for l in range(3)]

    key = tuple(np.float64(s) for s in scales)
    if key not in _CACHE:
        _CACHE[key] = build_nc(scales)
    nc = _CACHE[key]

    shared = {
        "idn": np.eye(128, dtype=np.float32),
        "biota": np.tile(np.arange(NW, dtype=np.float32), (16, 1)),
        "w1a": np.asarray(inputs["W1"], np.float32)[0:128],
        "w1b": np.asarray(inputs["W1"], np.float32)[128:256],
        "b1": np.asarray(inputs["b1"], np.float32).reshape(128, 1),
        "w2": np.asarray(inputs["W2"], np.float32),
        "b2": np.asarray(inputs["b2"], np.float32).reshape(64, 1),
        "w3": np.asarray(inputs["W3"], np.float32),
        "b3r": np.tile(np.asarray(inputs["b3"], np.float32), (16, 1)),
    }
    for l in range(3):
        shared[f"wl{l}"] = np.asarray(inputs[f"Wl{l+1}"], np.float32)
        shared[f"wr{l}"] = np.asarray(inputs[f"Wr{l+1}"], np.float32)
        shared[f"bl{l}"] = np.asarray(inputs[f"bl{l+1}"], np.float32).reshape(128, 1)
        shared[f"pw{l}"] = pws[l].reshape(128, 1)

    in_maps = []
    for c in range(NCORES):
        m = dict(shared)
        m["xn"] = xn[c * GPC:(c + 1) * GPC]
        m["at"] = at[c * GPC:(c + 1) * GPC]
        in_maps.append(m)

    trace = bool(os.environ.get("BASS_KERNEL_TRACE"))
    res = run_bass_kernel_spmd(nc, in_maps, list(range(NCORES)), trace=trace)
    if res.exec_time_ns is not None:
        LAST_EXEC_NS = res.exec_time_ns
    out = np.concatenate([np.asarray(res.results[i]["out"])
                          for i in range(NCORES)], axis=0)
    return out[:B].astype(np.float32)


if __name__ == "__main__":
    nc = build_nc([0.1, 0.1, 0.1])
    print("built ok; instructions:",
          sum(len(bb.instructions) for f in nc.m.functions for bb in f.blocks))

